# revision 8
# baseline (speedup 1.0000x reference)
"""GQA attention (B=4, S=1024, H=4096, 32 q heads / 8 kv heads, rotary) on 8 trn2 cores.

Sharding: DP4 x TP2. Core c = 2*b + j handles batch b with kv-head half j:
  - column-parallel wq/wk/wv (16 q heads / 4 kv heads per core)
  - row-parallel wo -> partial [S, H] outputs, host sums core pairs.

v2: all-bf16 matmul pipeline (2 elem/cycle rhs streaming), single-pass
projections with the full x resident in SBUF (no stash/qspill DRAM round
trips), v computed directly in natural [t, d] layout (no PE transposes),
host-packed DMA-contiguous weight blobs, and a software pipeline that
interleaves each q-head's scores matmuls into the next head's projection so
the ACT-engine exp stream (softmax) hides completely under PE matmul work.

Per-core dataflow:
  kT[d, t] = wk.T @ xT; v[t, d] = xT.T @ wv (both SBUF-resident, bf16)
  per q-head block cb: qT = wq_cb.T @ xT + rope ->
    scoresT[t,s] = kT.T @ qT (interleaved into proj of cb+1); exp on ACT;
    denom = ones.T @ expT (PE column-sum); 1/denom bcast via gpsimd;
    oT[d,s] = (v.T @ expT) * inv
  out = oT.T @ wo (bf16 operands, fp32 accumulate/out).
"""

import numpy as np

B = 4
S = 1024
H = 4096
D = 128
HQ = 32
HKV = 8
G = 4
NCORES = 8
QC = 2048  # q cols per core
KC = 512  # k cols per core
VC = 512  # v cols per core
COH = 2048  # wo rows per core
ROPE_BASE = 10000.0

NKO = H // 128  # 32 contraction tiles
KOC = 4  # ko tiles per x chunk
NCH = NKO // KOC  # 8 chunks

_CACHE = {}


def _build(reps=1):
    import concourse.tile as tile
    from concourse import bacc, mybir

    fp32 = mybir.dt.float32
    bf16 = mybir.dt.bfloat16

    nc = bacc.Bacc(None, target_bir_lowering=False)

    if reps != 1:
        # shape depends on reps so each variant gets a distinct HLO hash
        # (the jax-level neff cache would otherwise reuse the reps=1 NEFF)
        nc.dram_tensor("repstag", [1, 16 * reps], fp32, kind="ExternalInput")

    # host-packed, per-partition-contiguous blobs (bf16)
    xp_d = nc.dram_tensor("xp", [NCH, 128, KOC, S], bf16, kind="ExternalInput")
    wqp_d = nc.dram_tensor("wqp", [16, 128, NKO, 128], bf16, kind="ExternalInput")
    wkp_d = nc.dram_tensor("wkp", [4, 128, NKO, 128], bf16, kind="ExternalInput")
    wvp_d = nc.dram_tensor("wvp", [NCH, 128, KOC, VC], bf16, kind="ExternalInput")
    wop_d = nc.dram_tensor("wop", [8, 2, 128, 8, 512], bf16, kind="ExternalInput")
    aq_d = nc.dram_tensor("ropeAq", [D, S], fp32, kind="ExternalInput")
    bq_d = nc.dram_tensor("ropeBq", [D, S], fp32, kind="ExternalInput")
    ak_d = nc.dram_tensor("ropeAk", [D, S], fp32, kind="ExternalInput")
    bk_d = nc.dram_tensor("ropeBk", [D, S], fp32, kind="ExternalInput")
    out_d = nc.dram_tensor("out", [S, H], fp32, kind="ExternalOutput")
    out_r = out_d.rearrange("(tb p) h -> tb p h", p=128)  # [8, 128, 4096]

    with tile.TileContext(nc) as tc, nc.allow_low_precision(
        reason="bf16 matmul pipeline"
    ):
      for _rep in range(reps):
        with (
            tc.tile_pool(name="persist", bufs=1) as persist,
            tc.tile_pool(name="konst", bufs=1) as konst,
        ):
            kT = persist.tile([128, HKV // 2, S], bf16)  # [128, 4, 1024]
            v = persist.tile([128, S // 128, VC], bf16)  # [128, 8, 512]
            ones_f = konst.tile([128, 128], fp32)
            nc.vector.memset(ones_f[:], 1.0)
            ones = konst.tile([128, 128], bf16)
            nc.vector.tensor_copy(ones[:], ones_f[:])

            with (
                tc.tile_pool(name="ot", bufs=1) as opool,
                tc.tile_pool(name="xt", bufs=1) as xpool,
            ):
                oT = opool.tile([128, 16, S], bf16)  # 32 KiB/part

                # load the full x (64 KiB/partition, 8 chunks)
                xts = []
                for ch in range(NCH):
                    xt = xpool.tile([128, KOC, S], bf16, tag=f"xt{ch}",
                                    name=f"xt{ch}")
                    nc.sync.dma_start(xt[:], xp_d[ch])
                    xts.append(xt)

                def rope_evict(epool, raw_ps, Am, Bm, out_ap, th):
                    """out = raw*Am + swap128(raw)*Bm  (raw in PSUM, fp32)."""
                    ts_ = slice(th * 512, th * 512 + 512)
                    raw = epool.tile([128, 512], fp32, tag="raw", name="raw")
                    nc.vector.tensor_copy(raw[:], raw_ps[:])
                    t1 = epool.tile([128, 512], fp32, tag="t1", name="t1")
                    nc.vector.tensor_mul(t1[:], raw_ps[:], Am[:, ts_])
                    sw = epool.tile([128, 512], fp32, tag="sw", name="sw")
                    nc.sync.dma_start(sw[0:64, :], raw[64:128, :])
                    nc.sync.dma_start(sw[64:128, :], raw[0:64, :])
                    t2 = epool.tile([128, 512], fp32, tag="t2", name="t2")
                    nc.vector.tensor_mul(t2[:], sw[:], Bm[:, ts_])
                    nc.vector.tensor_add(out_ap, t1[:], t2[:])

                def proj_block(wt, ps_pool, interleave=None):
                    """64 accumulating matmuls -> psA/psB [128, 512] fp32.

                    interleave: list of 0-arg closures (scores MMs) emitted
                    between contraction steps to pace the ACT exp stream.
                    """
                    psA = ps_pool.tile([128, 512], fp32, tag="ps", name="psA")
                    psB = ps_pool.tile([128, 512], fp32, tag="ps", name="psB")
                    for ko in range(NKO):
                        xt = xts[ko // KOC]
                        j = ko % KOC
                        nc.tensor.matmul(
                            psA[:], wt[:, ko, :], xt[:, j, 0:512],
                            start=(ko == 0), stop=(ko == NKO - 1),
                        )
                        nc.tensor.matmul(
                            psB[:], wt[:, ko, :], xt[:, j, 512:1024],
                            start=(ko == 0), stop=(ko == NKO - 1),
                        )
                        if interleave and ko >= 11 and ko % 2 == 1:
                            interleave.pop(0)()
                    return psA, psB

                # ---- k projections (4 blocks) ----
                with (
                    tc.tile_pool(name="mapsk", bufs=1) as mpoolk,
                    tc.tile_pool(name="wtk", bufs=2) as wpool,
                    tc.tile_pool(name="evk", bufs=2) as epool,
                    tc.tile_pool(name="psk", bufs=3, space="PSUM") as pspool,
                ):
                    mapk = {}
                    for nm, dram in (("Ak", ak_d), ("Bk", bk_d)):
                        mt = mpoolk.tile([128, S], fp32, tag=nm, name=nm)
                        nc.sync.dma_start(mt[:], dram[:])
                        mapk[nm] = mt
                    wk_next = [None]

                    def load_wk(cb):
                        wt = wpool.tile([128, NKO, 128], bf16, tag="wt",
                                        name=f"wk{cb}")
                        nc.sync.dma_start(wt[:], wkp_d[cb])
                        return wt

                    wk_next[0] = load_wk(0)
                    for cb in range(4):
                        wt = wk_next[0]
                        psA, psB = proj_block(wt, pspool)
                        if cb + 1 < 4:
                            wk_next[0] = load_wk(cb + 1)
                        for th, ps in ((0, psA), (1, psB)):
                            ts_ = slice(th * 512, th * 512 + 512)
                            rope_evict(epool, ps, mapk["Ak"], mapk["Bk"],
                                       kT[:, cb, ts_], th)

                # ---- v (natural layout, 8 PSUM banks) ----
                with (
                    tc.tile_pool(name="wtv", bufs=2) as wvpool,
                    tc.tile_pool(name="psv", bufs=8, space="PSUM") as psvpool,
                ):
                    banks = [
                        psvpool.tile([128, VC], fp32, tag="psv", name=f"psv{tb}")
                        for tb in range(8)
                    ]
                    for ch in range(NCH):
                        wvc = wvpool.tile([128, KOC, VC], bf16, tag="wv",
                                          name=f"wv{ch}")
                        nc.sync.dma_start(wvc[:], wvp_d[ch])
                        for j in range(KOC):
                            for tb in range(8):
                                nc.tensor.matmul(
                                    banks[tb][:],
                                    xts[ch][:, j, tb * 128:(tb + 1) * 128],
                                    wvc[:, j, :],
                                    start=(ch == 0 and j == 0),
                                    stop=(ch == NCH - 1 and j == KOC - 1),
                                )
                    for tb in range(8):
                        nc.vector.tensor_copy(v[:, tb, :], banks[tb][:])

                # ---- q blocks softwarepipelined with attention ----
                with (
                    tc.tile_pool(name="mapsq", bufs=1) as mpoolq,
                    tc.tile_pool(name="wtq", bufs=2) as wqpool,
                    tc.tile_pool(name="evq", bufs=2) as epool,
                    tc.tile_pool(name="qbuf", bufs=2) as qpool,
                    tc.tile_pool(name="ex", bufs=2) as expool,
                    tc.tile_pool(name="sm", bufs=2) as smpool,
                    tc.tile_pool(name="psq", bufs=2, space="PSUM") as psqpool,
                    tc.tile_pool(name="pssc", bufs=4, space="PSUM") as pssc,
                    tc.tile_pool(name="psden", bufs=1, space="PSUM") as psden,
                    tc.tile_pool(name="pso", bufs=1, space="PSUM") as psopool,
                ):
                    mapq = {}
                    for nm, dram in (("Aq", aq_d), ("Bq", bq_d)):
                        mt = mpoolq.tile([128, S], fp32, tag=nm, name=nm)
                        nc.sync.dma_start(mt[:], dram[:])
                        mapq[nm] = mt

                    wq_next = [None]

                    def load_wq(cb):
                        wt = wqpool.tile([128, NKO, 128], bf16, tag="wt",
                                         name=f"wq{cb}")
                        nc.sync.dma_start(wt[:], wqp_d[cb])
                        return wt

                    wq_next[0] = load_wq(0)

                    def make_scores(cb, qt):
                        """16 closures: scores MM + exp for (cb, sh, tb).
                        Returns (closures, expT tiles per sh)."""
                        h = cb // 4
                        exps = [
                            expool.tile([128, 8, 512], bf16, tag=f"expT{sh}",
                                        name=f"expT{cb}_{sh}")
                            for sh in range(2)
                        ]
                        closures = []
                        for sh in range(2):
                            ss = slice(sh * 512, sh * 512 + 512)
                            for tb in range(8):
                                def emit(sh=sh, ss=ss, tb=tb):
                                    psc = pssc.tile([128, 512], fp32,
                                                    tag="psc", name="psc")
                                    nc.tensor.matmul(
                                        psc[:],
                                        kT[:, h, tb * 128:(tb + 1) * 128],
                                        qt[:, ss],
                                        start=True, stop=True,
                                    )
                                    nc.scalar.activation(
                                        exps[sh][:, tb], psc[:],
                                        mybir.ActivationFunctionType.Exp,
                                    )
                                closures.append(emit)
                        return closures, exps

                    def emit_tails(cb, exps, interleave):
                        """softmax denom + attn@v for head-block cb."""
                        h = cb // 4
                        for sh in range(2):
                            ss = slice(sh * 512, sh * 512 + 512)
                            expT = exps[sh]
                            pden = psden.tile([128, 512], fp32, tag="pd",
                                              name="pd")
                            for tb in range(8):
                                nc.tensor.matmul(
                                    pden[:], ones[:], expT[:, tb],
                                    start=(tb == 0), stop=(tb == 7),
                                )
                                if interleave and tb % 2 == 1:
                                    interleave.pop(0)()
                            invb = smpool.tile([128, 512], fp32, tag="invb",
                                               name="invb")
                            nc.vector.reciprocal_approx_fast(invb[:], pden[:])
                            po = psopool.tile([128, 512], fp32, tag="po",
                                              name="po")
                            for tb in range(8):
                                nc.tensor.matmul(
                                    po[:],
                                    v[:, tb, h * 128:(h + 1) * 128],
                                    expT[:, tb],
                                    start=(tb == 0), stop=(tb == 7),
                                )
                                if interleave and tb % 2 == 1:
                                    interleave.pop(0)()
                            nc.vector.tensor_mul(oT[:, cb, ss], po[:], invb[:])

                    pending_scores = []
                    pending_exps = None
                    for cb in range(16):
                        wt = wq_next[0]
                        psA, psB = proj_block(wt, psqpool,
                                              interleave=pending_scores)
                        if cb + 1 < 16:
                            wq_next[0] = load_wq(cb + 1)
                        qt = qpool.tile([128, S], bf16, tag="qt", name="qt")
                        for th, ps in ((0, psA), (1, psB)):
                            ts_ = slice(th * 512, th * 512 + 512)
                            rope_evict(epool, ps, mapq["Aq"], mapq["Bq"],
                                       qt[:, ts_], th)
                        if cb > 0:
                            emit_tails(cb - 1, pending_exps, pending_scores)
                        assert not pending_scores
                        pending_scores, pending_exps = make_scores(cb, qt)

                    # epilogue: last head's scores + tails
                    for c in pending_scores:
                        c()
                    emit_tails(15, pending_exps, [])

                # ---- phase 3: out = oT.T @ wo ----
                with (
                    tc.tile_pool(name="wot", bufs=2) as wopool,
                    tc.tile_pool(name="outp", bufs=2) as outpool,
                    tc.tile_pool(name="psout", bufs=3, space="PSUM") as psout,
                ):
                    def load_wo_strip(hh, half):
                        wot = wopool.tile([128, 8, 512], bf16,
                                          tag=f"wo{half}", name=f"wo{half}")
                        nc.sync.dma_start(wot[:], wop_d[hh, half])
                        return wot

                    wo_next = [load_wo_strip(0, 0)]
                    wotB_first = load_wo_strip(0, 1)
                    for hh in range(8):
                        hs = slice(hh * 512, hh * 512 + 512)
                        wotA = wo_next[0]
                        wotB = wotB_first if hh == 0 else load_wo_strip(hh, 1)
                        for tb in range(8):
                            pso_ = psout.tile([128, 512], fp32, tag="pso",
                                              name="pso_")
                            for co in range(8):
                                nc.tensor.matmul(
                                    pso_[:],
                                    oT[:, co, tb * 128:(tb + 1) * 128],
                                    wotA[:, co, :],
                                    start=(co == 0), stop=False,
                                )
                            if tb == 0 and hh < 7:
                                wo_next[0] = load_wo_strip(hh + 1, 0)
                            for co in range(8, 16):
                                nc.tensor.matmul(
                                    pso_[:],
                                    oT[:, co, tb * 128:(tb + 1) * 128],
                                    wotB[:, co - 8, :],
                                    start=False, stop=(co == 15),
                                )
                            ot = outpool.tile([128, 512], fp32, tag="ot",
                                              name="ot")
                            nc.vector.tensor_copy(ot[:], pso_[:])
                            nc.sync.dma_start(out_r[tb, :, hs], ot[:])

    nc.compile()
    return nc


def _host_prep(x, wq, wk, wv, wo, start_pos):
    import ml_dtypes

    bf16 = ml_dtypes.bfloat16
    x = np.asarray(x, dtype=np.float32)
    wq = np.asarray(wq, dtype=np.float32)
    wk = np.asarray(wk, dtype=np.float32)
    wv = np.asarray(wv, dtype=np.float32)
    wo = np.asarray(wo, dtype=np.float32)
    sp = int(np.asarray(start_pos))

    perm = np.concatenate([np.arange(0, 128, 2), np.arange(1, 128, 2)])

    def pack_proj(w):
        # w: [H, C] -> [C/128, 128p, NKO, 128c] with rope perm on cols
        C = w.shape[1]
        r = w.reshape(NKO, 128, C // 128, 128)[:, :, :, perm]
        return np.ascontiguousarray(r.transpose(2, 1, 0, 3)).astype(bf16)

    def pack_v(w):
        # w: [H, VC] -> [NCH, 128p, KOC, VC] (no perm)
        r = w.reshape(NCH, KOC, 128, VC)
        return np.ascontiguousarray(r.transpose(0, 2, 1, 3)).astype(bf16)

    def pack_wo(w):
        # w: [COH, H] -> [8hh, 2half, 128p, 8co, 512]; wo row =
        # half*1024 + co*128 + p, col = hh*512 + c
        r = w.reshape(2, 8, 128, 8, 512)
        return np.ascontiguousarray(r.transpose(3, 0, 2, 1, 4)).astype(bf16)

    def pack_x(xb):
        # xb: [S, H] -> xT[H, S] -> [NCH, 128p, KOC, S]
        xT = np.ascontiguousarray(xb.T).reshape(NCH, KOC, 128, S)
        return np.ascontiguousarray(xT.transpose(0, 2, 1, 3)).astype(bf16)

    inv_freq = 1.0 / (ROPE_BASE ** (np.arange(0, D, 2, dtype=np.float32) / D))
    t = np.arange(sp, sp + S, dtype=np.float32)
    freqs = t[None, :] * inv_freq[:, None]  # [64, S]
    sin, cos = np.sin(freqs), np.cos(freqs)
    A = np.concatenate([sin, sin], axis=0).astype(np.float32)  # [128, S]
    Bm = np.concatenate([-cos, cos], axis=0).astype(np.float32)
    scale = np.float32(1.0 / np.sqrt(np.float32(D)))
    maps = {
        "ropeAq": np.ascontiguousarray(A * scale),
        "ropeBq": np.ascontiguousarray(Bm * scale),
        "ropeAk": np.ascontiguousarray(A),
        "ropeBk": np.ascontiguousarray(Bm),
    }

    # weights are shared across batches: pack once per tp half
    wpacks = []
    for j in range(2):
        wpacks.append({
            "wqp": pack_proj(wq[:, j * QC:(j + 1) * QC]),
            "wkp": pack_proj(wk[:, j * KC:(j + 1) * KC]),
            "wvp": pack_v(wv[:, j * VC:(j + 1) * VC]),
            "wop": pack_wo(wo[j * COH:(j + 1) * COH, :]),
        })
    xpacks = [pack_x(x[b]) for b in range(B)]

    in_maps = []
    for c in range(NCORES):
        b, j = divmod(c, 2)
        im = {"xp": xpacks[b]}
        im.update(wpacks[j])
        im.update(maps)
        in_maps.append(im)
    return in_maps


def kernel(x, wq, wk, wv, wo, start_pos=0, _trace=False):
    from concourse.bass_utils import run_bass_kernel_spmd

    if "nc" not in _CACHE:
        _CACHE["nc"] = _build()
    nc = _CACHE["nc"]

    in_maps = _host_prep(x, wq, wk, wv, wo, start_pos)
    res = run_bass_kernel_spmd(nc, in_maps, core_ids=list(range(NCORES)), trace=_trace)
    _CACHE["last_result"] = res

    out = np.empty((B, S, H), dtype=np.float32)
    for b in range(B):
        out[b] = res.results[2 * b]["out"] + res.results[2 * b + 1]["out"]
    return out


# revision 11
# speedup vs baseline: 1.1068x; 1.1068x over previous
"""GQA attention (B=4, S=1024, H=4096, 32 q heads / 8 kv heads, rotary) on 8 trn2 cores.

Sharding: DP4 x TP2. Core c = 2*b + j handles batch b with kv-head half j:
  - column-parallel wq/wk/wv (16 q heads / 4 kv heads per core)
  - row-parallel wo -> partial [S, H] outputs, host sums core pairs.

v2: all-bf16 matmul pipeline (2 elem/cycle rhs streaming), single-pass
projections with the full x resident in SBUF (no stash/qspill DRAM round
trips), v computed directly in natural [t, d] layout (no PE transposes),
host-packed DMA-contiguous weight blobs, and a software pipeline that
interleaves each q-head's scores matmuls into the next head's projection so
the ACT-engine exp stream (softmax) hides completely under PE matmul work.

Per-core dataflow:
  kT[d, t] = wk.T @ xT; v[t, d] = xT.T @ wv (both SBUF-resident, bf16)
  per q-head block cb: qT = wq_cb.T @ xT + rope ->
    scoresT[t,s] = kT.T @ qT (interleaved into proj of cb+1); exp on ACT;
    denom = ones.T @ expT (PE column-sum); 1/denom bcast via gpsimd;
    oT[d,s] = (v.T @ expT) * inv
  out = oT.T @ wo (bf16 operands, fp32 accumulate/out).
"""

import numpy as np

B = 4
S = 1024
H = 4096
D = 128
HQ = 32
HKV = 8
G = 4
NCORES = 8
QC = 2048  # q cols per core
KC = 512  # k cols per core
VC = 512  # v cols per core
COH = 2048  # wo rows per core
ROPE_BASE = 10000.0

NKO = H // 128  # 32 contraction tiles
KOC = 4  # ko tiles per x chunk
NCH = NKO // KOC  # 8 chunks

_CACHE = {}


def _build(reps=1):
    import concourse.tile as tile
    from concourse import bacc, mybir

    fp32 = mybir.dt.float32
    bf16 = mybir.dt.bfloat16

    nc = bacc.Bacc(None, target_bir_lowering=False)

    if reps != 1:
        # shape depends on reps so each variant gets a distinct HLO hash
        # (the jax-level neff cache would otherwise reuse the reps=1 NEFF)
        nc.dram_tensor("repstag", [1, 16 * reps], fp32, kind="ExternalInput")

    # host-packed, per-partition-contiguous blobs (bf16)
    xp_d = nc.dram_tensor("xp", [NCH, 128, KOC, S], bf16, kind="ExternalInput")
    wqp_d = nc.dram_tensor("wqp", [16, 128, NKO, 128], bf16, kind="ExternalInput")
    wkp_d = nc.dram_tensor("wkp", [4, 128, NKO, 128], bf16, kind="ExternalInput")
    wvp_d = nc.dram_tensor("wvp", [NCH, 128, KOC, VC], bf16, kind="ExternalInput")
    wop_d = nc.dram_tensor("wop", [8, 2, 128, 8, 512], bf16, kind="ExternalInput")
    aq_d = nc.dram_tensor("ropeAq", [D, S], fp32, kind="ExternalInput")
    bq_d = nc.dram_tensor("ropeBq", [D, S], fp32, kind="ExternalInput")
    ak_d = nc.dram_tensor("ropeAk", [D, S], fp32, kind="ExternalInput")
    bk_d = nc.dram_tensor("ropeBk", [D, S], fp32, kind="ExternalInput")
    out_d = nc.dram_tensor("out", [S, H], fp32, kind="ExternalOutput")
    out_r = out_d.rearrange("(tb p) h -> tb p h", p=128)  # [8, 128, 4096]

    with tile.TileContext(nc) as tc, nc.allow_low_precision(
        reason="bf16 matmul pipeline"
    ):
      for _rep in range(reps):
        with (
            tc.tile_pool(name="persist", bufs=1) as persist,
            tc.tile_pool(name="konst", bufs=1) as konst,
        ):
            kT = persist.tile([128, HKV // 2, S], bf16)  # [128, 4, 1024]
            v = persist.tile([128, S // 128, VC], bf16)  # [128, 8, 512]
            fp16 = mybir.dt.float16
            ones_f = konst.tile([128, 128], fp32)
            nc.vector.memset(ones_f[:], 1.0)
            ones_h = konst.tile([128, 128], fp16)
            nc.vector.tensor_copy(ones_h[:], ones_f[:])

            with (
                tc.tile_pool(name="ot", bufs=1) as opool,
                tc.tile_pool(name="xt", bufs=1) as xpool,
            ):
                oT = opool.tile([128, 16, S], bf16)  # 32 KiB/part

                # load the full x (64 KiB/partition, 8 chunks)
                xts = []
                for ch in range(NCH):
                    xt = xpool.tile([128, KOC, S], bf16, tag=f"xt{ch}",
                                    name=f"xt{ch}")
                    nc.sync.dma_start(xt[:], xp_d[ch])
                    xts.append(xt)

                def rope_evict(epool, raw_ps, Am, Bm, out_ap, th):
                    """out = raw*Am + swap128(raw)*Bm  (raw in PSUM, fp32)."""
                    ts_ = slice(th * 512, th * 512 + 512)
                    raw = epool.tile([128, 512], fp32, tag="raw", name="raw")
                    nc.vector.tensor_copy(raw[:], raw_ps[:])
                    t1 = epool.tile([128, 512], fp32, tag="t1", name="t1")
                    nc.vector.tensor_mul(t1[:], raw_ps[:], Am[:, ts_])
                    sw = epool.tile([128, 512], fp32, tag="sw", name="sw")
                    nc.sync.dma_start(sw[0:64, :], raw[64:128, :])
                    nc.sync.dma_start(sw[64:128, :], raw[0:64, :])
                    t2 = epool.tile([128, 512], fp32, tag="t2", name="t2")
                    nc.vector.tensor_mul(t2[:], sw[:], Bm[:, ts_])
                    nc.vector.tensor_add(out_ap, t1[:], t2[:])

                def proj_block(wt, ps_pool, interleave=None):
                    """64 accumulating matmuls -> psA/psB [128, 512] fp32.

                    interleave: list of 0-arg closures (scores MMs) emitted
                    between contraction steps to pace the ACT exp stream.
                    """
                    psA = ps_pool.tile([128, 512], fp32, tag="ps", name="psA")
                    psB = ps_pool.tile([128, 512], fp32, tag="ps", name="psB")
                    for ko in range(NKO):
                        xt = xts[ko // KOC]
                        j = ko % KOC
                        nc.tensor.matmul(
                            psA[:], wt[:, ko, :], xt[:, j, 0:512],
                            start=(ko == 0), stop=(ko == NKO - 1),
                        )
                        nc.tensor.matmul(
                            psB[:], wt[:, ko, :], xt[:, j, 512:1024],
                            start=(ko == 0), stop=(ko == NKO - 1),
                        )
                        if interleave and ko >= 11 and ko % 2 == 1:
                            interleave.pop(0)()
                    return psA, psB

                # ---- v first (natural layout, 8 PSUM banks): its matmuls
                # stream chunk-by-chunk right behind the x DMAs, so the PE
                # starts ~3 us in instead of waiting for the whole x load ----
                with (
                    tc.tile_pool(name="wtv", bufs=2) as wvpool,
                    tc.tile_pool(name="psv", bufs=8, space="PSUM") as psvpool,
                ):
                    banks = [
                        psvpool.tile([128, VC], fp32, tag="psv", name=f"psv{tb}")
                        for tb in range(8)
                    ]
                    for ch in range(NCH):
                        wvc = wvpool.tile([128, KOC, VC], bf16, tag="wv",
                                          name=f"wv{ch}")
                        nc.sync.dma_start(wvc[:], wvp_d[ch])
                        for j in range(KOC):
                            for tb in range(8):
                                nc.tensor.matmul(
                                    banks[tb][:],
                                    xts[ch][:, j, tb * 128:(tb + 1) * 128],
                                    wvc[:, j, :],
                                    start=(ch == 0 and j == 0),
                                    stop=(ch == NCH - 1 and j == KOC - 1),
                                )
                    for tb in range(8):
                        nc.vector.tensor_copy(v[:, tb, :], banks[tb][:])

                # ---- k projections (4 blocks) ----
                with (
                    tc.tile_pool(name="mapsk", bufs=1) as mpoolk,
                    tc.tile_pool(name="wtk", bufs=2) as wpool,
                    tc.tile_pool(name="evk", bufs=2) as epool,
                    tc.tile_pool(name="psk", bufs=3, space="PSUM") as pspool,
                ):
                    mapk = {}
                    for nm, dram in (("Ak", ak_d), ("Bk", bk_d)):
                        mt = mpoolk.tile([128, S], fp32, tag=nm, name=nm)
                        nc.sync.dma_start(mt[:], dram[:])
                        mapk[nm] = mt
                    wk_next = [None]

                    def load_wk(cb):
                        wt = wpool.tile([128, NKO, 128], bf16, tag="wt",
                                        name=f"wk{cb}")
                        nc.sync.dma_start(wt[:], wkp_d[cb])
                        return wt

                    wk_next[0] = load_wk(0)
                    for cb in range(4):
                        wt = wk_next[0]
                        psA, psB = proj_block(wt, pspool)
                        if cb + 1 < 4:
                            wk_next[0] = load_wk(cb + 1)
                        for th, ps in ((0, psA), (1, psB)):
                            ts_ = slice(th * 512, th * 512 + 512)
                            rope_evict(epool, ps, mapk["Ak"], mapk["Bk"],
                                       kT[:, cb, ts_], th)

                # ---- q blocks softwarepipelined with attention ----
                with (
                    tc.tile_pool(name="mapsq", bufs=1) as mpoolq,
                    tc.tile_pool(name="wtq", bufs=2) as wqpool,
                    tc.tile_pool(name="evq", bufs=2) as epool,
                    tc.tile_pool(name="qbuf", bufs=2) as qpool,
                    tc.tile_pool(name="ex", bufs=2) as expool,
                    tc.tile_pool(name="sm", bufs=2) as smpool,
                    tc.tile_pool(name="psq", bufs=2, space="PSUM") as psqpool,
                    tc.tile_pool(name="pssc", bufs=4, space="PSUM") as pssc,
                    tc.tile_pool(name="psden", bufs=1, space="PSUM") as psden,
                    tc.tile_pool(name="pso", bufs=1, space="PSUM") as psopool,
                ):
                    mapq = {}
                    for nm, dram in (("Aq", aq_d), ("Bq", bq_d)):
                        mt = mpoolq.tile([128, S], fp32, tag=nm, name=nm)
                        nc.sync.dma_start(mt[:], dram[:])
                        mapq[nm] = mt

                    wq_next = [None]

                    def load_wq(cb):
                        wt = wqpool.tile([128, NKO, 128], bf16, tag="wt",
                                         name=f"wq{cb}")
                        nc.sync.dma_start(wt[:], wqp_d[cb])
                        return wt

                    wq_next[0] = load_wq(0)

                    def make_scores(cb, qt):
                        """16 closures: scores MM + exp for (cb, sh, tb).
                        Returns (closures, expT tiles per sh)."""
                        h = cb // 4
                        exps = [
                            expool.tile([128, 8, 512], bf16, tag=f"expT{sh}",
                                        name=f"expT{cb}_{sh}")
                            for sh in range(2)
                        ]
                        closures = []
                        for sh in range(2):
                            ss = slice(sh * 512, sh * 512 + 512)
                            for tb in range(8):
                                def emit(sh=sh, ss=ss, tb=tb):
                                    psc = pssc.tile([128, 512], fp32,
                                                    tag="psc", name="psc")
                                    nc.tensor.matmul(
                                        psc[:],
                                        kT[:, h, tb * 128:(tb + 1) * 128],
                                        qt[:, ss],
                                        start=True, stop=True,
                                    )
                                    nc.scalar.activation(
                                        exps[sh][:, tb], psc[:],
                                        mybir.ActivationFunctionType.Exp,
                                    )
                                closures.append(emit)
                        return closures, exps

                    def emit_tails(cb, exps, interleave):
                        """softmax denom + attn@v for head-block cb."""
                        h = cb // 4
                        for sh in range(2):
                            ss = slice(sh * 512, sh * 512 + 512)
                            expT = exps[sh]
                            # denom: DVE tree-add over tb (saves 7 PE matmuls),
                            # then one ones-matmul for the partition sum
                            tt = []
                            for i in range(4):
                                t = smpool.tile([128, 512], fp16, tag=f"ta{i}",
                                                name=f"ta{i}")
                                nc.vector.tensor_add(t[:], expT[:, 2 * i],
                                                     expT[:, 2 * i + 1])
                                tt.append(t)
                                if interleave:
                                    interleave.pop(0)()
                            nc.vector.tensor_add(tt[0][:], tt[0][:], tt[1][:])
                            nc.vector.tensor_add(tt[2][:], tt[2][:], tt[3][:])
                            nc.vector.tensor_add(tt[0][:], tt[0][:], tt[2][:])
                            pden = psden.tile([128, 512], fp32, tag="pd",
                                              name="pd")
                            nc.tensor.matmul(pden[:], ones_h[:], tt[0][:],
                                             start=True, stop=True)
                            if interleave:
                                interleave.pop(0)()
                            invb = smpool.tile([128, 512], fp32, tag="invb",
                                               name="invb")
                            nc.vector.reciprocal_approx_fast(invb[:], pden[:])
                            po = psopool.tile([128, 512], fp32, tag="po",
                                              name="po")
                            for tb in range(8):
                                nc.tensor.matmul(
                                    po[:],
                                    v[:, tb, h * 128:(h + 1) * 128],
                                    expT[:, tb],
                                    start=(tb == 0), stop=(tb == 7),
                                )
                                if interleave and tb % 2 == 1:
                                    interleave.pop(0)()
                            nc.vector.tensor_mul(oT[:, cb, ss], po[:], invb[:])

                    pending_scores = []
                    pending_exps = None
                    for cb in range(16):
                        wt = wq_next[0]
                        psA, psB = proj_block(wt, psqpool,
                                              interleave=pending_scores)
                        if cb + 1 < 16:
                            wq_next[0] = load_wq(cb + 1)
                        qt = qpool.tile([128, S], bf16, tag="qt", name="qt")
                        for th, ps in ((0, psA), (1, psB)):
                            ts_ = slice(th * 512, th * 512 + 512)
                            rope_evict(epool, ps, mapq["Aq"], mapq["Bq"],
                                       qt[:, ts_], th)
                        if cb > 0:
                            emit_tails(cb - 1, pending_exps, pending_scores)
                        assert not pending_scores
                        pending_scores, pending_exps = make_scores(cb, qt)

                    # epilogue: last head's scores + tails
                    for c in pending_scores:
                        c()
                    emit_tails(15, pending_exps, [])

                # ---- phase 3: out = oT.T @ wo ----
                with (
                    tc.tile_pool(name="wot", bufs=2) as wopool,
                    tc.tile_pool(name="outp", bufs=2) as outpool,
                    tc.tile_pool(name="psout", bufs=3, space="PSUM") as psout,
                ):
                    def load_wo_strip(hh, half):
                        wot = wopool.tile([128, 8, 512], bf16,
                                          tag=f"wo{half}", name=f"wo{half}")
                        nc.sync.dma_start(wot[:], wop_d[hh, half])
                        return wot

                    wo_next = [load_wo_strip(0, 0)]
                    wotB_first = load_wo_strip(0, 1)
                    for hh in range(8):
                        hs = slice(hh * 512, hh * 512 + 512)
                        wotA = wo_next[0]
                        wotB = wotB_first if hh == 0 else load_wo_strip(hh, 1)
                        for tb in range(8):
                            pso_ = psout.tile([128, 512], fp32, tag="pso",
                                              name="pso_")
                            for co in range(8):
                                nc.tensor.matmul(
                                    pso_[:],
                                    oT[:, co, tb * 128:(tb + 1) * 128],
                                    wotA[:, co, :],
                                    start=(co == 0), stop=False,
                                )
                            if tb == 0 and hh < 7:
                                wo_next[0] = load_wo_strip(hh + 1, 0)
                            for co in range(8, 16):
                                nc.tensor.matmul(
                                    pso_[:],
                                    oT[:, co, tb * 128:(tb + 1) * 128],
                                    wotB[:, co - 8, :],
                                    start=False, stop=(co == 15),
                                )
                            ot = outpool.tile([128, 512], fp32, tag="ot",
                                              name="ot")
                            nc.vector.tensor_copy(ot[:], pso_[:])
                            nc.sync.dma_start(out_r[tb, :, hs], ot[:])

    nc.compile()
    return nc


def _host_prep(x, wq, wk, wv, wo, start_pos):
    import ml_dtypes

    bf16 = ml_dtypes.bfloat16
    x = np.asarray(x, dtype=np.float32)
    wq = np.asarray(wq, dtype=np.float32)
    wk = np.asarray(wk, dtype=np.float32)
    wv = np.asarray(wv, dtype=np.float32)
    wo = np.asarray(wo, dtype=np.float32)
    sp = int(np.asarray(start_pos))

    perm = np.concatenate([np.arange(0, 128, 2), np.arange(1, 128, 2)])

    def pack_proj(w):
        # w: [H, C] -> [C/128, 128p, NKO, 128c] with rope perm on cols
        C = w.shape[1]
        r = w.reshape(NKO, 128, C // 128, 128)[:, :, :, perm]
        return np.ascontiguousarray(r.transpose(2, 1, 0, 3)).astype(bf16)

    def pack_v(w):
        # w: [H, VC] -> [NCH, 128p, KOC, VC] (no perm)
        r = w.reshape(NCH, KOC, 128, VC)
        return np.ascontiguousarray(r.transpose(0, 2, 1, 3)).astype(bf16)

    def pack_wo(w):
        # w: [COH, H] -> [8hh, 2half, 128p, 8co, 512]; wo row =
        # half*1024 + co*128 + p, col = hh*512 + c
        r = w.reshape(2, 8, 128, 8, 512)
        return np.ascontiguousarray(r.transpose(3, 0, 2, 1, 4)).astype(bf16)

    def pack_x(xb):
        # xb: [S, H] -> xT[H, S] -> [NCH, 128p, KOC, S]
        xT = np.ascontiguousarray(xb.T).reshape(NCH, KOC, 128, S)
        return np.ascontiguousarray(xT.transpose(0, 2, 1, 3)).astype(bf16)

    inv_freq = 1.0 / (ROPE_BASE ** (np.arange(0, D, 2, dtype=np.float32) / D))
    t = np.arange(sp, sp + S, dtype=np.float32)
    freqs = t[None, :] * inv_freq[:, None]  # [64, S]
    sin, cos = np.sin(freqs), np.cos(freqs)
    A = np.concatenate([sin, sin], axis=0).astype(np.float32)  # [128, S]
    Bm = np.concatenate([-cos, cos], axis=0).astype(np.float32)
    scale = np.float32(1.0 / np.sqrt(np.float32(D)))
    maps = {
        "ropeAq": np.ascontiguousarray(A * scale),
        "ropeBq": np.ascontiguousarray(Bm * scale),
        "ropeAk": np.ascontiguousarray(A),
        "ropeBk": np.ascontiguousarray(Bm),
    }

    # weights are shared across batches: pack once per tp half
    wpacks = []
    for j in range(2):
        wpacks.append({
            "wqp": pack_proj(wq[:, j * QC:(j + 1) * QC]),
            "wkp": pack_proj(wk[:, j * KC:(j + 1) * KC]),
            "wvp": pack_v(wv[:, j * VC:(j + 1) * VC]),
            "wop": pack_wo(wo[j * COH:(j + 1) * COH, :]),
        })
    xpacks = [pack_x(x[b]) for b in range(B)]

    in_maps = []
    for c in range(NCORES):
        b, j = divmod(c, 2)
        im = {"xp": xpacks[b]}
        im.update(wpacks[j])
        im.update(maps)
        in_maps.append(im)
    return in_maps


def kernel(x, wq, wk, wv, wo, start_pos=0, _trace=False):
    from concourse.bass_utils import run_bass_kernel_spmd

    if "nc" not in _CACHE:
        _CACHE["nc"] = _build()
    nc = _CACHE["nc"]

    in_maps = _host_prep(x, wq, wk, wv, wo, start_pos)
    res = run_bass_kernel_spmd(nc, in_maps, core_ids=list(range(NCORES)), trace=_trace)
    _CACHE["last_result"] = res

    out = np.empty((B, S, H), dtype=np.float32)
    for b in range(B):
        out[b] = res.results[2 * b]["out"] + res.results[2 * b + 1]["out"]
    return out
